# revision 4
# baseline (speedup 1.0000x reference)
"""Cached single-head attention (B=4, S=KV=2048, D=2048) on 8 TRN2 NeuronCores.

Sharding: data-parallel over (batch, S-half) -> 8 shards. Each core:
  - projects K^T and V for its WHOLE batch (duplicated across the pair of
    cores sharing a batch; avoids cross-core communication),
  - projects Q^T for its own 1024 query rows,
  - computes scores^T = Kc^T-chunks (stationary) x Q^T (moving) so that the
    exp'd probabilities come out directly in the [t, s] layout the attention
    matmul needs as its stationary operand (no on-chip transpose at all),
  - softmax without max-subtraction (scores ~ N(0,1), exp can't overflow),
    denominator via ones-vector matmuls, applied to the PSUM output,
  - final round-half-even to 1e-4 via the +/- 1.5*2^23 float trick.

All matmuls run in bf16 (fp32 PSUM accumulate); K/V outputs are written from
the fp32 accumulators so the returned cache is near-exact.
"""

import os

import numpy as np
import ml_dtypes

import concourse.bass as bass
import concourse.mybir as mybir
import concourse.tile as tile
from concourse import bacc
from concourse.bass_utils import run_bass_kernel_spmd

F32 = mybir.dt.float32
BF16 = mybir.dt.bfloat16

B, S, KV, D = 4, 2048, 2048, 2048
TKV = KV + S          # total keys per batch
SH = S // 2           # query rows handled per core
N_CORES = 8
P = 128               # SBUF partitions

DO = D // P           # 16 contraction chunks for projections
EO = D // P           # 16 e-chunks (feature dim on partitions)
TC = TKV // P         # 32 key chunks
SCALE = float(1.0 / np.sqrt(np.float32(D)))
C_RND = 12582912.0    # 1.5 * 2**23: fp32 add/sub rounds to nearest int (half-even)

_CACHED_NC = None
last_results = None   # BassKernelResults of the most recent run (for test harness)


def _emit(tc_ctx, t):
    nc = tc_ctx.nc
    tc = tc_ctx

    with tc.tile_pool(name="resident", bufs=1) as resident:
        ones_bf = resident.tile([P, 1], BF16)
        nc.vector.memset(ones_bf[:], 1.0)
        qt_sb = resident.tile([P, EO, SH], BF16)       # Q^T  [e, s]
        l_row = resident.tile([1, SH], F32)            # softmax sums, row layout
        rl4 = resident.tile([P, SH // P], F32)         # 1e4 / l, per-partition layout

        # ---------------- Phase Q: Q^T[e, s] = Wq^T (stationary) x xq^T ------
        with (
            tc.tile_pool(name="phq", bufs=1) as phq,
            tc.tile_pool(name="phq_w", bufs=3) as phq_w,
            tc.tile_pool(name="psq", bufs=4, space="PSUM") as psq,
        ):
            xtq_sb = phq.tile([P, DO, SH], BF16)
            nc.sync.dma_start(
                out=xtq_sb[:],
                in_=t["xtq"].rearrange("(do p) s -> p do s", p=P),
            )
            wqt_re = t["wqt"].rearrange("(do p) e -> p do e", p=P)
            for ec in range(EO):
                wq_t = phq_w.tile([P, DO, P], BF16)
                nc.sync.dma_start(out=wq_t[:], in_=wqt_re[:, :, ec * P:(ec + 1) * P])
                for sc in range(SH // 512):
                    ps = psq.tile([P, 512], F32)
                    for dc in range(DO):
                        nc.tensor.matmul(
                            ps[:],
                            wq_t[:, dc, :],
                            xtq_sb[:, dc, sc * 512:(sc + 1) * 512],
                            start=(dc == 0),
                            stop=(dc == DO - 1),
                        )
                    nc.vector.tensor_copy(
                        qt_sb[:, ec, sc * 512:(sc + 1) * 512], ps[:]
                    )

        # ---------------- Phase K: K^T[e, t] = Wk^T (stationary) x x^T -------
        xt_re = t["xt"].rearrange("(do p) s -> p do s", p=P)
        kt_out_re = t["kt_out"].rearrange("(eo p) s -> p eo s", p=P)
        ktn_re = t["ktn"].rearrange("(eo p) s -> p eo s", p=P)
        with (
            tc.tile_pool(name="phk_w", bufs=1) as phk_w,
            tc.tile_pool(name="phk", bufs=2) as phk,
            tc.tile_pool(name="phk_st", bufs=3) as phk_st,
            tc.tile_pool(name="psk", bufs=4, space="PSUM") as psk,
        ):
            wk_sb = phk_w.tile([P, DO, D], BF16)
            nc.sync.dma_start(
                out=wk_sb[:], in_=t["wkt"].rearrange("(do p) e -> p do e", p=P)
            )
            for tb in range(S // 512):
                xt_blk = phk.tile([P, DO, 512], BF16)
                nc.sync.dma_start(
                    out=xt_blk[:], in_=xt_re[:, :, tb * 512:(tb + 1) * 512]
                )
                for ec in range(EO):
                    ps = psk.tile([P, 512], F32)
                    for dc in range(DO):
                        nc.tensor.matmul(
                            ps[:],
                            wk_sb[:, dc, ec * P:(ec + 1) * P],
                            xt_blk[:, dc, :],
                            start=(dc == 0),
                            stop=(dc == DO - 1),
                        )
                    kt_f = phk_st.tile([P, 512], F32)
                    nc.vector.tensor_copy(kt_f[:], ps[:])
                    nc.sync.dma_start(
                        out=kt_out_re[:, ec, tb * 512:(tb + 1) * 512], in_=kt_f[:]
                    )
                    kt_b = phk_st.tile([P, 512], BF16)
                    nc.vector.tensor_copy(kt_b[:], kt_f[:])
                    nc.sync.dma_start(
                        out=ktn_re[:, ec, tb * 512:(tb + 1) * 512], in_=kt_b[:]
                    )

        # ---------------- Phase V: V[t, e] = x^T chunk (stationary) x Wv^T ---
        v_out_re = t["v_out"].rearrange("(to p) e -> p to e", p=P)
        vn_re = t["vn"].rearrange("(to p) e -> p to e", p=P)
        with (
            tc.tile_pool(name="phv_w", bufs=1) as phv_w,
            tc.tile_pool(name="phv", bufs=2) as phv,
            tc.tile_pool(name="phv_st", bufs=3) as phv_st,
            tc.tile_pool(name="psv", bufs=4, space="PSUM") as psv,
        ):
            wv_sb = phv_w.tile([P, DO, D], BF16)
            nc.sync.dma_start(
                out=wv_sb[:], in_=t["wvt"].rearrange("(do p) e -> p do e", p=P)
            )
            for tb in range(S // 512):
                xt_blk2 = phv.tile([P, DO, 512], BF16)
                nc.sync.dma_start(
                    out=xt_blk2[:], in_=xt_re[:, :, tb * 512:(tb + 1) * 512]
                )
                for t128 in range(4):
                    to = tb * 4 + t128
                    for ec in range(D // 512):
                        ps = psv.tile([P, 512], F32)
                        for dc in range(DO):
                            nc.tensor.matmul(
                                ps[:],
                                xt_blk2[:, dc, t128 * P:(t128 + 1) * P],
                                wv_sb[:, dc, ec * 512:(ec + 1) * 512],
                                start=(dc == 0),
                                stop=(dc == DO - 1),
                            )
                        v_f = phv_st.tile([P, 512], F32)
                        nc.vector.tensor_copy(v_f[:], ps[:])
                        nc.sync.dma_start(
                            out=v_out_re[:, to, ec * 512:(ec + 1) * 512], in_=v_f[:]
                        )
                        v_b = phv_st.tile([P, 512], BF16)
                        nc.vector.tensor_copy(v_b[:], v_f[:])
                        nc.sync.dma_start(
                            out=vn_re[:, to, ec * 512:(ec + 1) * 512], in_=v_b[:]
                        )

        # ---------------- Phase A: scores^T -> exp -> P^T and row sums ------
        pkt_re = t["pkt"].rearrange("(eo p) s -> p eo s", p=P)
        with tc.tile_pool(name="pt_pool", bufs=1) as pt_pool:
            pt_sb = pt_pool.tile([P, TC, SH], BF16)    # P^T = exp(scores^T)

            with (
                tc.tile_pool(name="pha", bufs=3) as pha,
                tc.tile_pool(name="psa", bufs=3, space="PSUM") as psa,
                tc.tile_pool(name="psl", bufs=1, space="PSUM") as psl,
            ):
                ps_l = [psl.tile([1, 512], F32, name=f"psl{i}", tag=f"psl{i}")
                        for i in range(SH // 512)]
                for tcb in range(TC):
                    kct = pha.tile([P, EO, P], BF16)
                    if tcb < KV // P:
                        src = pkt_re[:, :, tcb * P:(tcb + 1) * P]
                    else:
                        src = ktn_re[:, :, (tcb - KV // P) * P:(tcb - KV // P + 1) * P]
                    nc.sync.dma_start(out=kct[:], in_=src)
                    for sc in range(SH // 512):
                        ps_st = psa.tile([P, 512], F32)
                        for ec in range(EO):
                            nc.tensor.matmul(
                                ps_st[:],
                                kct[:, ec, :],
                                qt_sb[:, ec, sc * 512:(sc + 1) * 512],
                                start=(ec == 0),
                                stop=(ec == EO - 1),
                            )
                        nc.scalar.activation(
                            pt_sb[:, tcb, sc * 512:(sc + 1) * 512],
                            ps_st[:],
                            mybir.ActivationFunctionType.Exp,
                            scale=SCALE,
                        )
                        # accumulate softmax denominators: ones^T x P^T chunk
                        nc.tensor.matmul(
                            ps_l[sc][:],
                            ones_bf[:, 0:1],
                            pt_sb[:, tcb, sc * 512:(sc + 1) * 512],
                            start=(tcb == 0),
                            stop=(tcb == TC - 1),
                        )
                for sc in range(SH // 512):
                    nc.vector.tensor_copy(l_row[:, sc * 512:(sc + 1) * 512], ps_l[sc][:])
            # transpose l_row [1, SH] -> [P, SH/P] via a DRAM bounce (DRAM APs
            # have no partition-layout constraints)
            nc.sync.dma_start(out=t["l_scratch"], in_=l_row[0:1, :])
            nc.sync.dma_start(
                out=rl4[:], in_=t["l_scratch"].rearrange("(g p) -> p g", p=P)
            )
            nc.vector.reciprocal(rl4[:], rl4[:])
            nc.vector.tensor_scalar(
                out=rl4[:], in0=rl4[:], scalar1=1.0e4, scalar2=None,
                op0=mybir.AluOpType.mult,
            )

            # ---------------- Phase B: out = (P^T)^T x Vc, scale + round ----
            pv_re = t["pv"].rearrange("(to p) e -> p to e", p=P)
            out_re = t["out"].rearrange("(so p) e -> p so e", p=P)
            with (
                tc.tile_pool(name="phb", bufs=2) as phb,
                tc.tile_pool(name="phb_st", bufs=3) as phb_st,
                tc.tile_pool(name="psb", bufs=3, space="PSUM") as psb,
            ):
                for ec in range(D // 512):
                    vc = phb.tile([P, TC, 512], BF16)
                    nc.sync.dma_start(
                        out=vc[:, 0:KV // P, :],
                        in_=pv_re[:, :, ec * 512:(ec + 1) * 512],
                    )
                    nc.sync.dma_start(
                        out=vc[:, KV // P:TC, :],
                        in_=vn_re[:, :, ec * 512:(ec + 1) * 512],
                    )
                    for ssub in range(SH // P):
                        ps_o = psb.tile([P, 512], F32)
                        for tcb in range(TC):
                            nc.tensor.matmul(
                                ps_o[:],
                                pt_sb[:, tcb, ssub * P:(ssub + 1) * P],
                                vc[:, tcb, :],
                                start=(tcb == 0),
                                stop=(tcb == TC - 1),
                            )
                        yb = phb_st.tile([P, 512], F32)
                        nc.vector.tensor_scalar(
                            out=yb[:], in0=ps_o[:],
                            scalar1=rl4[:, ssub:ssub + 1], scalar2=C_RND,
                            op0=mybir.AluOpType.mult, op1=mybir.AluOpType.add,
                        )
                        oo = phb_st.tile([P, 512], F32)
                        nc.vector.tensor_scalar(
                            out=oo[:], in0=yb[:],
                            scalar1=-C_RND, scalar2=1.0e-4,
                            op0=mybir.AluOpType.add, op1=mybir.AluOpType.mult,
                        )
                        nc.sync.dma_start(
                            out=out_re[:, ssub, ec * 512:(ec + 1) * 512], in_=oo[:]
                        )


def _build():
    nc = bacc.Bacc("TRN2", target_bir_lowering=False, debug=False,
                   num_devices=N_CORES)
    t = {}
    t["xt"] = nc.dram_tensor("xt", [D, S], BF16, kind="ExternalInput").ap()
    t["xtq"] = nc.dram_tensor("xtq", [D, SH], BF16, kind="ExternalInput").ap()
    t["wqt"] = nc.dram_tensor("wqt", [D, D], BF16, kind="ExternalInput").ap()
    t["wkt"] = nc.dram_tensor("wkt", [D, D], BF16, kind="ExternalInput").ap()
    t["wvt"] = nc.dram_tensor("wvt", [D, D], BF16, kind="ExternalInput").ap()
    t["pkt"] = nc.dram_tensor("pkt", [D, KV], BF16, kind="ExternalInput").ap()
    t["pv"] = nc.dram_tensor("pv", [KV, D], BF16, kind="ExternalInput").ap()
    t["kt_out"] = nc.dram_tensor("kt_out", [D, S], F32, kind="ExternalOutput").ap()
    t["v_out"] = nc.dram_tensor("v_out", [S, D], F32, kind="ExternalOutput").ap()
    t["out"] = nc.dram_tensor("out", [SH, D], F32, kind="ExternalOutput").ap()
    t["ktn"] = nc.dram_tensor("ktn", [D, S], BF16).ap()   # scratch
    t["vn"] = nc.dram_tensor("vn", [S, D], BF16).ap()     # scratch
    t["l_scratch"] = nc.dram_tensor("l_scratch", [SH], F32).ap()  # scratch

    with tile.TileContext(nc) as tc_ctx:
        _emit(tc_ctx, t)
    nc.finalize()
    return nc


def _get_nc():
    global _CACHED_NC
    if _CACHED_NC is None:
        _CACHED_NC = _build()
    return _CACHED_NC


def kernel(x, past_k, past_v, Wq, Wk, Wv):
    global last_results
    x = np.asarray(x, dtype=np.float32)
    past_k = np.asarray(past_k, dtype=np.float32)
    past_v = np.asarray(past_v, dtype=np.float32)
    bf = ml_dtypes.bfloat16

    wqt = np.ascontiguousarray(np.asarray(Wq, np.float32).T).astype(bf)
    wkt = np.ascontiguousarray(np.asarray(Wk, np.float32).T).astype(bf)
    wvt = np.ascontiguousarray(np.asarray(Wv, np.float32).T).astype(bf)
    xt_b = [np.ascontiguousarray(x[b].T).astype(bf) for b in range(B)]
    pkt_b = [np.ascontiguousarray(past_k[b].T).astype(bf) for b in range(B)]
    pv_b = [np.ascontiguousarray(past_v[b]).astype(bf) for b in range(B)]

    in_maps = []
    for c in range(N_CORES):
        b, h = divmod(c, 2)
        in_maps.append({
            "xt": xt_b[b],
            "xtq": np.ascontiguousarray(xt_b[b][:, h * SH:(h + 1) * SH]),
            "wqt": wqt, "wkt": wkt, "wvt": wvt,
            "pkt": pkt_b[b], "pv": pv_b[b],
        })

    trace = bool(os.environ.get("BASS_TRACE"))
    res = run_bass_kernel_spmd(_get_nc(), in_maps, list(range(N_CORES)),
                               trace=trace)
    last_results = res

    out = np.empty((B, S, D), np.float32)
    k_new = np.empty((B, S, D), np.float32)
    v_new = np.empty((B, S, D), np.float32)
    for c in range(N_CORES):
        b, h = divmod(c, 2)
        out[b, h * SH:(h + 1) * SH] = res.results[c]["out"]
        if h == 0:
            k_new[b] = res.results[c]["kt_out"].T
            v_new[b] = res.results[c]["v_out"]
    cache_k = np.concatenate([past_k, k_new], axis=1)
    cache_v = np.concatenate([past_v, v_new], axis=1)
    return out, cache_k, cache_v


# revision 7
# speedup vs baseline: 1.0579x; 1.0579x over previous
"""Cached single-head attention (B=4, S=KV=2048, D=2048) on 8 TRN2 NeuronCores.

Sharding: data-parallel over (batch, S-half) -> 8 shards. Each core:
  - projects Q^T, K^T and V for its own 1024 rows of its batch,
  - pair-wise AllGather ([0,1],[2,3],...) of the bf16 K^T/V halves so both
    cores of a batch see the full new K/V for attention,
  - computes scores^T = Kc^T-chunks (stationary) x Q^T (moving) so that the
    exp'd probabilities come out directly in the [t, s] layout the attention
    matmul needs as its stationary operand (no on-chip transpose at all),
  - softmax without max-subtraction (scores ~ N(0,1), exp can't overflow),
    denominator via ones-vector matmuls, applied to the PSUM output,
  - final round-half-even to 1e-4 via the +/- 1.5*2^23 float trick.

All matmuls run in bf16 (fp32 PSUM accumulate); K/V outputs are written from
the fp32 accumulators so the returned cache is near-exact.
"""

import os

import numpy as np
import ml_dtypes

import concourse.bass as bass
import concourse.mybir as mybir
import concourse.tile as tile
from concourse import bacc
from concourse.bass_utils import run_bass_kernel_spmd

F32 = mybir.dt.float32
BF16 = mybir.dt.bfloat16

B, S, KV, D = 4, 2048, 2048, 2048
TKV = KV + S          # total keys per batch
SH = S // 2           # rows handled per core (queries AND new K/V rows)
N_CORES = 8
P = 128               # SBUF partitions

DO = D // P           # 16 contraction chunks for projections
EO = D // P           # 16 e-chunks (feature dim on partitions)
TC = TKV // P         # 32 key chunks
KVC = KV // P         # 16 past-key chunks
SHC = SH // P         # 8 chunks per half
SCALE = float(1.0 / np.sqrt(np.float32(D)))
C_RND = 12582912.0    # 1.5 * 2**23: fp32 add/sub rounds to nearest int (half-even)

REPLICA_GROUPS = [[0, 1], [2, 3], [4, 5], [6, 7]]

_CACHED_NC = None
last_results = None   # BassKernelResults of the most recent run (for test harness)


def _emit(tc_ctx, t):
    nc = tc_ctx.nc
    tc = tc_ctx

    with tc.tile_pool(name="resident", bufs=1) as resident:
        ones_bf = resident.tile([P, 1], BF16)
        nc.vector.memset(ones_bf[:], 1.0)
        qt_sb = resident.tile([P, EO, SH], BF16)       # Q^T  [e, s]
        rl4 = resident.tile([P, SHC], F32)             # 1e4 / l, per-partition

        xtq_cm = tc.tile_pool(name="xtq_pool", bufs=1)
        xtq_pool = xtq_cm.__enter__()
        xtq_sb = xtq_pool.tile([P, DO, SH], BF16)      # x^T half [d, t]
        nc.sync.dma_start(
            out=xtq_sb[:], in_=t["xtq"].rearrange("(do p) s -> p do s", p=P)
        )

        # ---------------- Phase Q: Q^T[e, s] = Wq^T (stationary) x xq^T ------
        wqt_re = t["wqt"].rearrange("(do p) e -> p do e", p=P)
        with (
            tc.tile_pool(name="phq_w", bufs=3) as phq_w,
            tc.tile_pool(name="psq", bufs=4, space="PSUM") as psq,
        ):
            for ec in range(EO):
                wq_t = phq_w.tile([P, DO, P], BF16)
                nc.sync.dma_start(out=wq_t[:], in_=wqt_re[:, :, ec * P:(ec + 1) * P])
                for sc in range(SH // 512):
                    ps = psq.tile([P, 512], F32)
                    for dc in range(DO):
                        nc.tensor.matmul(
                            ps[:],
                            wq_t[:, dc, :],
                            xtq_sb[:, dc, sc * 512:(sc + 1) * 512],
                            start=(dc == 0),
                            stop=(dc == DO - 1),
                        )
                    nc.vector.tensor_copy(
                        qt_sb[:, ec, sc * 512:(sc + 1) * 512], ps[:]
                    )

        # ---------------- Phase K: K^T[e, t-half] = Wk^T (stat) x xq^T -------
        kt_out_re = t["kt_out"].rearrange("(eo p) s -> p eo s", p=P)
        ktn_re = t["ktn"].rearrange("(eo p) s -> p eo s", p=P)
        wkt_re = t["wkt"].rearrange("(do p) e -> p do e", p=P)
        with (
            tc.tile_pool(name="phk_w", bufs=3) as phk_w,
            tc.tile_pool(name="phk_st", bufs=3) as phk_st,
            tc.tile_pool(name="psk", bufs=4, space="PSUM") as psk,
        ):
            for ec in range(EO):
                wk_t = phk_w.tile([P, DO, P], BF16)
                nc.sync.dma_start(out=wk_t[:], in_=wkt_re[:, :, ec * P:(ec + 1) * P])
                for tb in range(SH // 512):
                    ps = psk.tile([P, 512], F32)
                    for dc in range(DO):
                        nc.tensor.matmul(
                            ps[:],
                            wk_t[:, dc, :],
                            xtq_sb[:, dc, tb * 512:(tb + 1) * 512],
                            start=(dc == 0),
                            stop=(dc == DO - 1),
                        )
                    kt_f = phk_st.tile([P, 512], F32)
                    nc.vector.tensor_copy(kt_f[:], ps[:])
                    nc.sync.dma_start(
                        out=kt_out_re[:, ec, tb * 512:(tb + 1) * 512], in_=kt_f[:]
                    )
                    kt_b = phk_st.tile([P, 512], BF16)
                    nc.vector.tensor_copy(kt_b[:], kt_f[:])
                    nc.sync.dma_start(
                        out=ktn_re[:, ec, tb * 512:(tb + 1) * 512], in_=kt_b[:]
                    )

        # gather pair halves of K^T (overlaps with phase V below)
        nc.gpsimd.collective_compute(
            "AllGather", mybir.AluOpType.bypass,
            replica_groups=REPLICA_GROUPS,
            ins=[t["ktn"]], outs=[t["ktn_g"]],
        )

        # ---------------- Phase V: V[t-half, e] = xq^T chunk (stat) x Wv^T ---
        v_out_re = t["v_out"].rearrange("(to p) e -> p to e", p=P)
        vn_re = t["vn"].rearrange("(to p) e -> p to e", p=P)
        wvt_re = t["wvt"].rearrange("(do p) e -> p do e", p=P)
        with (
            tc.tile_pool(name="phv_w", bufs=2) as phv_w,
            tc.tile_pool(name="phv_st", bufs=3) as phv_st,
            tc.tile_pool(name="psv", bufs=4, space="PSUM") as psv,
        ):
            for ec in range(D // 512):
                wv_t = phv_w.tile([P, DO, 512], BF16)
                nc.sync.dma_start(
                    out=wv_t[:], in_=wvt_re[:, :, ec * 512:(ec + 1) * 512]
                )
                for to in range(SHC):
                    ps = psv.tile([P, 512], F32)
                    for dc in range(DO):
                        nc.tensor.matmul(
                            ps[:],
                            xtq_sb[:, dc, to * P:(to + 1) * P],
                            wv_t[:, dc, :],
                            start=(dc == 0),
                            stop=(dc == DO - 1),
                        )
                    v_f = phv_st.tile([P, 512], F32)
                    nc.vector.tensor_copy(v_f[:], ps[:])
                    nc.sync.dma_start(
                        out=v_out_re[:, to, ec * 512:(ec + 1) * 512], in_=v_f[:]
                    )
                    v_b = phv_st.tile([P, 512], BF16)
                    nc.vector.tensor_copy(v_b[:], v_f[:])
                    nc.sync.dma_start(
                        out=vn_re[:, to, ec * 512:(ec + 1) * 512], in_=v_b[:]
                    )

        # gather pair halves of V (overlaps with past-key scoring below)
        nc.gpsimd.collective_compute(
            "AllGather", mybir.AluOpType.bypass,
            replica_groups=REPLICA_GROUPS,
            ins=[t["vn"]], outs=[t["vn_g"]],
        )
        xtq_cm.__exit__(None, None, None)

        # ---------------- Phase A: scores^T -> exp -> P^T and row sums ------
        pkt_re = t["pkt"].rearrange("(eo p) s -> p eo s", p=P)
        ktn_g_re = [
            t["ktn_g"][a].rearrange("(eo p) s -> p eo s", p=P) for a in range(2)
        ]
        with tc.tile_pool(name="pt_pool", bufs=1) as pt_pool:
            pt_sb = pt_pool.tile([P, TC, SH], BF16)    # P^T = exp(scores^T)
            l_row = pt_pool.tile([1, SH], F32)         # softmax sums, row layout

            with (
                tc.tile_pool(name="pha", bufs=3) as pha,
                tc.tile_pool(name="psa", bufs=3, space="PSUM") as psa,
                tc.tile_pool(name="psl", bufs=1, space="PSUM") as psl,
            ):
                ps_l = [psl.tile([1, 512], F32, name=f"psl{i}", tag=f"psl{i}")
                        for i in range(SH // 512)]
                for tcb in range(TC):
                    kct = pha.tile([P, EO, P], BF16)
                    if tcb < KVC:
                        src = pkt_re[:, :, tcb * P:(tcb + 1) * P]
                    else:
                        tn = tcb - KVC
                        a, tl = divmod(tn, SHC)
                        src = ktn_g_re[a][:, :, tl * P:(tl + 1) * P]
                    nc.sync.dma_start(out=kct[:], in_=src)
                    for sc in range(SH // 512):
                        ps_st = psa.tile([P, 512], F32)
                        for ec in range(EO):
                            nc.tensor.matmul(
                                ps_st[:],
                                kct[:, ec, :],
                                qt_sb[:, ec, sc * 512:(sc + 1) * 512],
                                start=(ec == 0),
                                stop=(ec == EO - 1),
                            )
                        nc.scalar.activation(
                            pt_sb[:, tcb, sc * 512:(sc + 1) * 512],
                            ps_st[:],
                            mybir.ActivationFunctionType.Exp,
                            scale=SCALE,
                        )
                        # accumulate softmax denominators: ones^T x P^T chunk
                        nc.tensor.matmul(
                            ps_l[sc][:],
                            ones_bf[:, 0:1],
                            pt_sb[:, tcb, sc * 512:(sc + 1) * 512],
                            start=(tcb == 0),
                            stop=(tcb == TC - 1),
                        )
                for sc in range(SH // 512):
                    nc.vector.tensor_copy(l_row[:, sc * 512:(sc + 1) * 512], ps_l[sc][:])
            # transpose l_row [1, SH] -> [P, SH/P] via a DRAM bounce (DRAM APs
            # have no partition-layout constraints)
            nc.sync.dma_start(out=t["l_scratch"], in_=l_row[0:1, :])
            nc.sync.dma_start(
                out=rl4[:], in_=t["l_scratch"].rearrange("(g p) -> p g", p=P)
            )
            nc.vector.reciprocal(rl4[:], rl4[:])
            nc.vector.tensor_scalar(
                out=rl4[:], in0=rl4[:], scalar1=1.0e4, scalar2=None,
                op0=mybir.AluOpType.mult,
            )

            # ---------------- Phase B: out = (P^T)^T x Vc, scale + round ----
            pv_re = t["pv"].rearrange("(to p) e -> p to e", p=P)
            vn_g_re = [
                t["vn_g"][a].rearrange("(to p) e -> p to e", p=P) for a in range(2)
            ]
            out_re = t["out"].rearrange("(so p) e -> p so e", p=P)
            with (
                tc.tile_pool(name="phb", bufs=2) as phb,
                tc.tile_pool(name="phb_st", bufs=3) as phb_st,
                tc.tile_pool(name="psb", bufs=3, space="PSUM") as psb,
            ):
                for ec in range(D // 512):
                    vc = phb.tile([P, TC, 512], BF16)
                    nc.sync.dma_start(
                        out=vc[:, 0:KVC, :],
                        in_=pv_re[:, :, ec * 512:(ec + 1) * 512],
                    )
                    for a in range(2):
                        nc.sync.dma_start(
                            out=vc[:, KVC + a * SHC:KVC + (a + 1) * SHC, :],
                            in_=vn_g_re[a][:, :, ec * 512:(ec + 1) * 512],
                        )
                    for ssub in range(SHC):
                        ps_o = psb.tile([P, 512], F32)
                        for tcb in range(TC):
                            nc.tensor.matmul(
                                ps_o[:],
                                pt_sb[:, tcb, ssub * P:(ssub + 1) * P],
                                vc[:, tcb, :],
                                start=(tcb == 0),
                                stop=(tcb == TC - 1),
                            )
                        yb = phb_st.tile([P, 512], F32)
                        nc.vector.tensor_scalar(
                            out=yb[:], in0=ps_o[:],
                            scalar1=rl4[:, ssub:ssub + 1], scalar2=C_RND,
                            op0=mybir.AluOpType.mult, op1=mybir.AluOpType.add,
                        )
                        oo = phb_st.tile([P, 512], F32)
                        nc.vector.tensor_scalar(
                            out=oo[:], in0=yb[:],
                            scalar1=-C_RND, scalar2=1.0e-4,
                            op0=mybir.AluOpType.add, op1=mybir.AluOpType.mult,
                        )
                        nc.sync.dma_start(
                            out=out_re[:, ssub, ec * 512:(ec + 1) * 512], in_=oo[:]
                        )


def _build():
    nc = bacc.Bacc("TRN2", target_bir_lowering=False, debug=False,
                   num_devices=N_CORES)
    t = {}
    t["xtq"] = nc.dram_tensor("xtq", [D, SH], BF16, kind="ExternalInput").ap()
    t["wqt"] = nc.dram_tensor("wqt", [D, D], BF16, kind="ExternalInput").ap()
    t["wkt"] = nc.dram_tensor("wkt", [D, D], BF16, kind="ExternalInput").ap()
    t["wvt"] = nc.dram_tensor("wvt", [D, D], BF16, kind="ExternalInput").ap()
    t["pkt"] = nc.dram_tensor("pkt", [D, KV], BF16, kind="ExternalInput").ap()
    t["pv"] = nc.dram_tensor("pv", [KV, D], BF16, kind="ExternalInput").ap()
    t["kt_out"] = nc.dram_tensor("kt_out", [D, SH], F32, kind="ExternalOutput").ap()
    t["v_out"] = nc.dram_tensor("v_out", [SH, D], F32, kind="ExternalOutput").ap()
    t["out"] = nc.dram_tensor("out", [SH, D], F32, kind="ExternalOutput").ap()
    t["ktn"] = nc.dram_tensor("ktn", [D, SH], BF16).ap()      # scratch, own half
    t["vn"] = nc.dram_tensor("vn", [SH, D], BF16).ap()        # scratch, own half
    t["ktn_g"] = nc.dram_tensor("ktn_g", [2, D, SH], BF16).ap()   # gathered
    t["vn_g"] = nc.dram_tensor("vn_g", [2, SH, D], BF16).ap()     # gathered
    t["l_scratch"] = nc.dram_tensor("l_scratch", [SH], F32).ap()  # scratch

    with tile.TileContext(nc) as tc_ctx:
        _emit(tc_ctx, t)
    nc.finalize()
    return nc


def _get_nc():
    global _CACHED_NC
    if _CACHED_NC is None:
        _CACHED_NC = _build()
    return _CACHED_NC


def kernel(x, past_k, past_v, Wq, Wk, Wv):
    global last_results
    x = np.asarray(x, dtype=np.float32)
    past_k = np.asarray(past_k, dtype=np.float32)
    past_v = np.asarray(past_v, dtype=np.float32)
    bf = ml_dtypes.bfloat16

    wqt = np.ascontiguousarray(np.asarray(Wq, np.float32).T).astype(bf)
    wkt = np.ascontiguousarray(np.asarray(Wk, np.float32).T).astype(bf)
    wvt = np.ascontiguousarray(np.asarray(Wv, np.float32).T).astype(bf)
    xt_b = [np.ascontiguousarray(x[b].T).astype(bf) for b in range(B)]
    pkt_b = [np.ascontiguousarray(past_k[b].T).astype(bf) for b in range(B)]
    pv_b = [np.ascontiguousarray(past_v[b]).astype(bf) for b in range(B)]

    in_maps = []
    for c in range(N_CORES):
        b, h = divmod(c, 2)
        in_maps.append({
            "xtq": np.ascontiguousarray(xt_b[b][:, h * SH:(h + 1) * SH]),
            "wqt": wqt, "wkt": wkt, "wvt": wvt,
            "pkt": pkt_b[b], "pv": pv_b[b],
        })

    trace = bool(os.environ.get("BASS_TRACE"))
    res = run_bass_kernel_spmd(_get_nc(), in_maps, list(range(N_CORES)),
                               trace=trace)
    last_results = res

    out = np.empty((B, S, D), np.float32)
    k_new = np.empty((B, S, D), np.float32)
    v_new = np.empty((B, S, D), np.float32)
    for c in range(N_CORES):
        b, h = divmod(c, 2)
        out[b, h * SH:(h + 1) * SH] = res.results[c]["out"]
        k_new[b, h * SH:(h + 1) * SH] = res.results[c]["kt_out"].T
        v_new[b, h * SH:(h + 1) * SH] = res.results[c]["v_out"]
    cache_k = np.concatenate([past_k, k_new], axis=1)
    cache_v = np.concatenate([past_v, v_new], axis=1)
    return out, cache_k, cache_v


# revision 9
# speedup vs baseline: 1.0611x; 1.0031x over previous
"""Cached single-head attention (B=4, S=KV=2048, D=2048) on 8 TRN2 NeuronCores.

Sharding: data-parallel over (batch, S-half) -> 8 shards. Each core:
  - projects Q^T, K^T and V for its own 1024 rows of its batch,
  - pair-wise AllGather ([0,1],[2,3],...) of the bf16 K^T/V halves so both
    cores of a batch see the full new K/V for attention,
  - computes scores^T = Kc^T-chunks (stationary) x Q^T (moving) so that the
    exp'd probabilities come out directly in the [t, s] layout the attention
    matmul needs as its stationary operand (no on-chip transpose at all),
  - softmax without max-subtraction (scores ~ N(0,1), exp can't overflow),
    denominator via ones-vector matmuls, applied to the PSUM output,
  - final round-half-even to 1e-4 via the +/- 1.5*2^23 float trick.

All matmuls run in bf16 (fp32 PSUM accumulate); K/V outputs are written from
the fp32 accumulators so the returned cache is near-exact.
"""

import os

import numpy as np
import ml_dtypes

import concourse.bass as bass
import concourse.mybir as mybir
import concourse.tile as tile
from concourse import bacc
from concourse.bass_utils import run_bass_kernel_spmd

F32 = mybir.dt.float32
BF16 = mybir.dt.bfloat16

B, S, KV, D = 4, 2048, 2048, 2048
TKV = KV + S          # total keys per batch
SH = S // 2           # rows handled per core (queries AND new K/V rows)
N_CORES = 8
P = 128               # SBUF partitions

DO = D // P           # 16 contraction chunks for projections
EO = D // P           # 16 e-chunks (feature dim on partitions)
TC = TKV // P         # 32 key chunks
KVC = KV // P         # 16 past-key chunks
SHC = SH // P         # 8 chunks per half
SCALE = float(1.0 / np.sqrt(np.float32(D)))
C_RND = 12582912.0    # 1.5 * 2**23: fp32 add/sub rounds to nearest int (half-even)

REPLICA_GROUPS = [[0, 1], [2, 3], [4, 5], [6, 7]]

_CACHED_NC = None
last_results = None   # BassKernelResults of the most recent run (for test harness)


def _emit(tc_ctx, t):
    nc = tc_ctx.nc
    tc = tc_ctx

    kt_out_re = t["kt_out"].rearrange("(eo p) s -> p eo s", p=P)
    ktn_re = t["ktn"].rearrange("(eo p) s -> p eo s", p=P)
    v_out_re = t["v_out"].rearrange("(to p) e -> p to e", p=P)
    vn_re = t["vn"].rearrange("(to p) e -> p to e", p=P)
    wqt_re = t["wqt"].rearrange("(do p) e -> p do e", p=P)
    wkt_re = t["wkt"].rearrange("(do p) e -> p do e", p=P)
    wvt_re = t["wvt"].rearrange("(do p) e -> p do e", p=P)
    pkt_re = t["pkt"].rearrange("(eo p) s -> p eo s", p=P)
    pv_re = t["pv"].rearrange("(to p) e -> p to e", p=P)
    out_re = t["out"].rearrange("(so p) e -> p so e", p=P)
    ktn_g_re = [t["ktn_g"][a].rearrange("(eo p) s -> p eo s", p=P) for a in range(2)]
    vn_g_re = [t["vn_g"][a].rearrange("(to p) e -> p to e", p=P) for a in range(2)]

    with (
        tc.tile_pool(name="resident", bufs=1) as resident,
        tc.tile_pool(name="pha", bufs=4) as pha,        # kct stream (phase A)
    ):
        ones_bf = resident.tile([P, 1], BF16)
        nc.vector.memset(ones_bf[:], 1.0)
        qt_sb = resident.tile([P, EO, SH], BF16)       # Q^T  [e, s]
        rl4 = resident.tile([P, SHC], F32)             # 1e4 / l, per-partition

        with tc.tile_pool(name="xtq_pool", bufs=1) as xtq_pool:
            xtq_sb = xtq_pool.tile([P, DO, SH], BF16)  # x^T half [d, t]
            nc.sync.dma_start(
                out=xtq_sb[:], in_=t["xtq"].rearrange("(do p) s -> p do s", p=P)
            )

            # -------- Phase K: K^T[e, t-half] = Wk^T (stationary) x xq^T -----
            with (
                tc.tile_pool(name="phk_w", bufs=3) as phk_w,
                tc.tile_pool(name="phk_st", bufs=4) as phk_st,
                tc.tile_pool(name="psp", bufs=4, space="PSUM") as psp,
            ):
                for ec in range(EO):
                    wk_t = phk_w.tile([P, DO, P], BF16)
                    nc.sync.dma_start(
                        out=wk_t[:], in_=wkt_re[:, :, ec * P:(ec + 1) * P]
                    )
                    for tb in range(SH // 512):
                        ps = psp.tile([P, 512], F32)
                        for dc in range(DO):
                            nc.tensor.matmul(
                                ps[:],
                                wk_t[:, dc, :],
                                xtq_sb[:, dc, tb * 512:(tb + 1) * 512],
                                start=(dc == 0),
                                stop=(dc == DO - 1),
                            )
                        kt_f = phk_st.tile([P, 512], F32)
                        nc.vector.tensor_copy(kt_f[:], ps[:])
                        nc.sync.dma_start(
                            out=kt_out_re[:, ec, tb * 512:(tb + 1) * 512], in_=kt_f[:]
                        )
                        kt_b = phk_st.tile([P, 512], BF16)
                        nc.vector.tensor_copy(kt_b[:], kt_f[:])
                        nc.sync.dma_start(
                            out=ktn_re[:, ec, tb * 512:(tb + 1) * 512], in_=kt_b[:]
                        )

                # gather pair halves of K^T early (overlaps phases V and Q)
                nc.gpsimd.collective_compute(
                    "AllGather", mybir.AluOpType.bypass,
                    replica_groups=REPLICA_GROUPS,
                    ins=[t["ktn"]], outs=[t["ktn_g"]],
                )

                # ---- Phase V (pools nested inside K scope: no WAR stall) ----
                with (
                    tc.tile_pool(name="phv_w", bufs=2) as phv_w,
                    tc.tile_pool(name="phv_st", bufs=4) as phv_st,
                ):
                    for ec in range(D // 512):
                        wv_t = phv_w.tile([P, DO, 512], BF16)
                        nc.sync.dma_start(
                            out=wv_t[:], in_=wvt_re[:, :, ec * 512:(ec + 1) * 512]
                        )
                        for to in range(SHC):
                            ps = psp.tile([P, 512], F32)
                            for dc in range(DO):
                                nc.tensor.matmul(
                                    ps[:],
                                    xtq_sb[:, dc, to * P:(to + 1) * P],
                                    wv_t[:, dc, :],
                                    start=(dc == 0),
                                    stop=(dc == DO - 1),
                                )
                            v_f = phv_st.tile([P, 512], F32)
                            nc.vector.tensor_copy(v_f[:], ps[:])
                            nc.sync.dma_start(
                                out=v_out_re[:, to, ec * 512:(ec + 1) * 512],
                                in_=v_f[:],
                            )
                            v_b = phv_st.tile([P, 512], BF16)
                            nc.vector.tensor_copy(v_b[:], v_f[:])
                            nc.sync.dma_start(
                                out=vn_re[:, to, ec * 512:(ec + 1) * 512], in_=v_b[:]
                            )

                    # gather pair halves of V (overlaps phase Q + past-key A)
                    nc.gpsimd.collective_compute(
                        "AllGather", mybir.AluOpType.bypass,
                        replica_groups=REPLICA_GROUPS,
                        ins=[t["vn"]], outs=[t["vn_g"]],
                    )

                    # ---- Phase Q (nested; wq stream prefetches during V) ----
                    with (
                        tc.tile_pool(name="phq_w", bufs=3) as phq_w,
                    ):
                        for ec in range(EO):
                            wq_t = phq_w.tile([P, DO, P], BF16)
                            nc.sync.dma_start(
                                out=wq_t[:], in_=wqt_re[:, :, ec * P:(ec + 1) * P]
                            )
                            for sc in range(SH // 512):
                                ps = psp.tile([P, 512], F32)
                                for dc in range(DO):
                                    nc.tensor.matmul(
                                        ps[:],
                                        wq_t[:, dc, :],
                                        xtq_sb[:, dc, sc * 512:(sc + 1) * 512],
                                        start=(dc == 0),
                                        stop=(dc == DO - 1),
                                    )
                                nc.vector.tensor_copy(
                                    qt_sb[:, ec, sc * 512:(sc + 1) * 512], ps[:]
                                )

        # ---------------- Phase A: scores^T -> exp -> P^T and row sums ------
        with tc.tile_pool(name="pt_pool", bufs=1) as pt_pool:
            pt_sb = pt_pool.tile([P, TC, SH], BF16)    # P^T = exp(scores^T)
            l_row = pt_pool.tile([1, SH], F32)         # softmax sums, row layout

            with (
                tc.tile_pool(name="psa", bufs=3, space="PSUM") as psa,
                tc.tile_pool(name="psl", bufs=1, space="PSUM") as psl,
            ):
                ps_l = [psl.tile([1, 512], F32, name=f"psl{i}", tag=f"psl{i}")
                        for i in range(SH // 512)]
                for tcb in range(TC):
                    kct = pha.tile([P, EO, P], BF16)
                    if tcb < KVC:
                        src = pkt_re[:, :, tcb * P:(tcb + 1) * P]
                    else:
                        tn = tcb - KVC
                        a, tl = divmod(tn, SHC)
                        src = ktn_g_re[a][:, :, tl * P:(tl + 1) * P]
                    nc.sync.dma_start(out=kct[:], in_=src)
                    for sc in range(SH // 512):
                        ps_st = psa.tile([P, 512], F32)
                        for ec in range(EO):
                            nc.tensor.matmul(
                                ps_st[:],
                                kct[:, ec, :],
                                qt_sb[:, ec, sc * 512:(sc + 1) * 512],
                                start=(ec == 0),
                                stop=(ec == EO - 1),
                            )
                        nc.scalar.activation(
                            pt_sb[:, tcb, sc * 512:(sc + 1) * 512],
                            ps_st[:],
                            mybir.ActivationFunctionType.Exp,
                            scale=SCALE,
                        )
                        # accumulate softmax denominators: ones^T x P^T chunk
                        nc.tensor.matmul(
                            ps_l[sc][:],
                            ones_bf[:, 0:1],
                            pt_sb[:, tcb, sc * 512:(sc + 1) * 512],
                            start=(tcb == 0),
                            stop=(tcb == TC - 1),
                        )
                for sc in range(SH // 512):
                    nc.vector.tensor_copy(
                        l_row[:, sc * 512:(sc + 1) * 512], ps_l[sc][:]
                    )
            # transpose l_row [1, SH] -> [P, SH/P] via a DRAM bounce (DRAM APs
            # have no partition-layout constraints)
            nc.sync.dma_start(out=t["l_scratch"], in_=l_row[0:1, :])
            nc.sync.dma_start(
                out=rl4[:], in_=t["l_scratch"].rearrange("(g p) -> p g", p=P)
            )
            nc.vector.reciprocal(rl4[:], rl4[:])
            nc.vector.tensor_scalar(
                out=rl4[:], in0=rl4[:], scalar1=1.0e4, scalar2=None,
                op0=mybir.AluOpType.mult,
            )

            # ---------------- Phase B: out = (P^T)^T x Vc, scale + round ----
            with (
                tc.tile_pool(name="phb", bufs=2) as phb,
                tc.tile_pool(name="phb_st", bufs=4) as phb_st,
                tc.tile_pool(name="psb", bufs=3, space="PSUM") as psb,
            ):
                for ec in range(D // 512):
                    vc = phb.tile([P, TC, 512], BF16)
                    nc.sync.dma_start(
                        out=vc[:, 0:KVC, :],
                        in_=pv_re[:, :, ec * 512:(ec + 1) * 512],
                    )
                    for a in range(2):
                        nc.sync.dma_start(
                            out=vc[:, KVC + a * SHC:KVC + (a + 1) * SHC, :],
                            in_=vn_g_re[a][:, :, ec * 512:(ec + 1) * 512],
                        )
                    for ssub in range(SHC):
                        ps_o = psb.tile([P, 512], F32)
                        for tcb in range(TC):
                            nc.tensor.matmul(
                                ps_o[:],
                                pt_sb[:, tcb, ssub * P:(ssub + 1) * P],
                                vc[:, tcb, :],
                                start=(tcb == 0),
                                stop=(tcb == TC - 1),
                            )
                        yb = phb_st.tile([P, 512], F32)
                        nc.vector.tensor_scalar(
                            out=yb[:], in0=ps_o[:],
                            scalar1=rl4[:, ssub:ssub + 1], scalar2=C_RND,
                            op0=mybir.AluOpType.mult, op1=mybir.AluOpType.add,
                        )
                        oo = phb_st.tile([P, 512], F32)
                        nc.vector.tensor_scalar(
                            out=oo[:], in0=yb[:],
                            scalar1=-C_RND, scalar2=1.0e-4,
                            op0=mybir.AluOpType.add, op1=mybir.AluOpType.mult,
                        )
                        nc.sync.dma_start(
                            out=out_re[:, ssub, ec * 512:(ec + 1) * 512], in_=oo[:]
                        )


def _build():
    nc = bacc.Bacc("TRN2", target_bir_lowering=False, debug=False,
                   num_devices=N_CORES)
    t = {}
    t["xtq"] = nc.dram_tensor("xtq", [D, SH], BF16, kind="ExternalInput").ap()
    t["wqt"] = nc.dram_tensor("wqt", [D, D], BF16, kind="ExternalInput").ap()
    t["wkt"] = nc.dram_tensor("wkt", [D, D], BF16, kind="ExternalInput").ap()
    t["wvt"] = nc.dram_tensor("wvt", [D, D], BF16, kind="ExternalInput").ap()
    t["pkt"] = nc.dram_tensor("pkt", [D, KV], BF16, kind="ExternalInput").ap()
    t["pv"] = nc.dram_tensor("pv", [KV, D], BF16, kind="ExternalInput").ap()
    t["kt_out"] = nc.dram_tensor("kt_out", [D, SH], F32, kind="ExternalOutput").ap()
    t["v_out"] = nc.dram_tensor("v_out", [SH, D], F32, kind="ExternalOutput").ap()
    t["out"] = nc.dram_tensor("out", [SH, D], F32, kind="ExternalOutput").ap()
    t["ktn"] = nc.dram_tensor("ktn", [D, SH], BF16).ap()      # scratch, own half
    t["vn"] = nc.dram_tensor("vn", [SH, D], BF16).ap()        # scratch, own half
    t["ktn_g"] = nc.dram_tensor("ktn_g", [2, D, SH], BF16).ap()   # gathered
    t["vn_g"] = nc.dram_tensor("vn_g", [2, SH, D], BF16).ap()     # gathered
    t["l_scratch"] = nc.dram_tensor("l_scratch", [SH], F32).ap()  # scratch

    with tile.TileContext(nc) as tc_ctx:
        _emit(tc_ctx, t)
    nc.finalize()
    return nc


def _get_nc():
    global _CACHED_NC
    if _CACHED_NC is None:
        _CACHED_NC = _build()
    return _CACHED_NC


def kernel(x, past_k, past_v, Wq, Wk, Wv):
    global last_results
    x = np.asarray(x, dtype=np.float32)
    past_k = np.asarray(past_k, dtype=np.float32)
    past_v = np.asarray(past_v, dtype=np.float32)
    bf = ml_dtypes.bfloat16

    wqt = np.ascontiguousarray(np.asarray(Wq, np.float32).T).astype(bf)
    wkt = np.ascontiguousarray(np.asarray(Wk, np.float32).T).astype(bf)
    wvt = np.ascontiguousarray(np.asarray(Wv, np.float32).T).astype(bf)
    xt_b = [np.ascontiguousarray(x[b].T).astype(bf) for b in range(B)]
    pkt_b = [np.ascontiguousarray(past_k[b].T).astype(bf) for b in range(B)]
    pv_b = [np.ascontiguousarray(past_v[b]).astype(bf) for b in range(B)]

    in_maps = []
    for c in range(N_CORES):
        b, h = divmod(c, 2)
        in_maps.append({
            "xtq": np.ascontiguousarray(xt_b[b][:, h * SH:(h + 1) * SH]),
            "wqt": wqt, "wkt": wkt, "wvt": wvt,
            "pkt": pkt_b[b], "pv": pv_b[b],
        })

    trace = bool(os.environ.get("BASS_TRACE"))
    res = run_bass_kernel_spmd(_get_nc(), in_maps, list(range(N_CORES)),
                               trace=trace)
    last_results = res

    out = np.empty((B, S, D), np.float32)
    k_new = np.empty((B, S, D), np.float32)
    v_new = np.empty((B, S, D), np.float32)
    for c in range(N_CORES):
        b, h = divmod(c, 2)
        out[b, h * SH:(h + 1) * SH] = res.results[c]["out"]
        k_new[b, h * SH:(h + 1) * SH] = res.results[c]["kt_out"].T
        v_new[b, h * SH:(h + 1) * SH] = res.results[c]["v_out"]
    cache_k = np.concatenate([past_k, k_new], axis=1)
    cache_v = np.concatenate([past_v, v_new], axis=1)
    return out, cache_k, cache_v
